# revision 10
# baseline (speedup 1.0000x reference)
"""Trainium2 Bass kernel for nn_ConvProjector (conv3x3 -> ReLU -> conv3x3 -> ReLU
-> adaptive-avg-pool upsample 32x32 -> 687x1024 -> 1x1 conv 256->24 + bias).

Strategy (v2):
  * Pool and 1x1 conv commute: reduce 256->24 channels at 32x32 first, then
    upsample only 24 channels.
  * W axis: 1024 = 32*32 -> pure replication via a 0/1 expansion matmul.
  * H axis: 687 from 32 -> 21/22-row runs; replicated rows via one
    stride-0-source DMA per chunk, averaged boundary rows via a second
    accumulating expansion matmul scaled 0.5.
  * Sharding: core k owns input rows 4k..4k+3 (+halos); no collectives.
  * v2 overlap work: 576-ch dim packed as 4 full K=128 chunks + one K=64
    chunk (no zero padding streamed); matmuls ordered by DMA arrival across
    two balanced HWDGE queues; PE warmed with dummy matmuls during the
    initial DMA wait; h-major (p = 24h + c) output partition layout makes
    the psr->expansion handoff a plain copy; back-end chunked in two so the
    second chunk's compute hides under the first chunk's output write.
Output is assembled on the host from the per-core (4, 24, 22, 1024) buffers.
"""
import sys

if '/opt/trn_rl_repo' not in sys.path:
    sys.path.insert(0, '/opt/trn_rl_repo')

import numpy as np

IN_C, MID_C, OUT_C = 576, 256, 24
H = W = 32
OUT_H, OUT_W = 687, 1024
NCORES = 8
P = 128
KC1 = 4           # full 128-channel input chunks for conv1 (plus one 64 chunk)
KC2 = 2           # 256/128 chunks for conv2 / 1x1
MC = 2            # 256/128 output-channel chunks for conv1/conv2
W36 = 36          # padded row width (2 zero cols each side)
RX, R1, R2 = 9, 7, 5          # x rows / h1 rows / h2 (=r) rows per core
XBLK = RX * W36               # 324  per-kc x block
XSLACK = 16                   # rhs overrun slack so N can pad to 256
N1 = 256                      # conv1 matmul N (covers the 7 h1 rows)
H1BLK = R1 * W36              # 252  per-mc h1 block
H1SLACK = 80
N2 = 180                      # conv2 matmul N (covers the 5 h2 rows)
NV2 = 176                     # valid h2 flat span per mc
RUN = 22                      # output rows per owned input row in core buffer

_prog_cache = {}


def _h_runs():
    i = np.arange(OUT_H)
    s = (i * H) // OUT_H
    t = np.searchsorted(s, np.arange(H + 1), side='left')
    return s, t


def _build_program():
    import concourse.bass as bass
    import concourse.bacc as bacc
    import concourse.mybir as mybir
    from concourse.tile import TileContext

    f32 = mybir.dt.float32
    f16 = mybir.dt.float16
    nc = bacc.Bacc("TRN2", target_bir_lowering=False, debug=False,
                   num_devices=NCORES)

    w1blk = KC1 * MC * P          # 1024 cols per conv1 tap tile
    w2blk = 3 * KC2 * MC * P      # 1536 cols per conv2 tap-triple tile

    xs_d = nc.dram_tensor("xs", [P, KC1 * XBLK + XSLACK], f16, kind="ExternalInput")
    xh_d = nc.dram_tensor("xh", [64, XBLK + XSLACK], f16, kind="ExternalInput")
    w1_d = nc.dram_tensor("w1p", [P, 9 * w1blk], f16, kind="ExternalInput")
    wh_d = nc.dram_tensor("whp", [64, 9 * MC * P], f16, kind="ExternalInput")
    w2_d = nc.dram_tensor("w2p", [P, 3 * w2blk], f16, kind="ExternalInput")
    wm_d = nc.dram_tensor("wmp", [P, KC2 * OUT_C + R1], f16, kind="ExternalInput")
    bb_d = nc.dram_tensor("bbp", [P, 5], f32, kind="ExternalInput")
    em_d = nc.dram_tensor("emp", [32, OUT_W], f16, kind="ExternalInput")
    out_d = nc.dram_tensor("outb", [4 * OUT_C * RUN, OUT_W], f16,
                           kind="ExternalOutput")

    Relu = mybir.ActivationFunctionType.Relu
    Ident = mybir.ActivationFunctionType.Identity

    with TileContext(nc) as tc:
        with (
            tc.tile_pool(name="sb", bufs=1) as sb,
            tc.tile_pool(name="ps", bufs=1, space="PSUM") as psp,
        ):
            wz_t = sb.tile([P, N1], f16)
            x_t = sb.tile([P, KC1 * XBLK + XSLACK], f16)
            xh_t = sb.tile([64, XBLK + XSLACK], f16)
            w1_ts = [sb.tile([P, w1blk], f16, tag=f"w1_{t}",
                             name=f"w1t{t}") for t in range(9)]
            wh_t = sb.tile([64, 9 * MC * P], f16)
            w2_ts = [sb.tile([P, w2blk], f16, tag=f"w2_{t}",
                             name=f"w2t{t}") for t in range(3)]
            wm_t = sb.tile([P, KC2 * OUT_C + R1], f16)
            bb_t = sb.tile([P, 5], f32)
            em_t = sb.tile([32, OUT_W], f16)
            h1_t = sb.tile([P, MC * H1BLK + H1SLACK], f16)
            h2_t = sb.tile([P, MC * NV2], f16)
            rt_t = sb.tile([32, R2 * OUT_C], f16)
            rw_t = sb.tile([P, OUT_W], f16)
            av_t = sb.tile([P, OUT_W], f16)

            # ---- input streams, balanced across the two HWDGE queues ----
            # sync:   xs, w1 taps 0/2/4/6/8, w2 triples 1,2       (~2.03 MB)
            # scalar: xh, biases, wr+mask, w1 taps 1/3/5/7, w1-kc4,
            #         em, w2 triple 0                             (~2.25 MB)
            nc.sync.dma_start(x_t[:], xs_d.ap())
            nc.scalar.dma_start(xh_t[:], xh_d.ap())
            nc.scalar.dma_start(bb_t[:], bb_d.ap())
            nc.scalar.dma_start(wm_t[:], wm_d.ap())
            for t in range(9):
                eng = nc.sync if t % 2 == 0 else nc.scalar
                eng.dma_start(
                    w1_ts[t][:],
                    bass.AP(w1_d, t * w1blk, [[9 * w1blk, P], [1, w1blk]]))
            nc.scalar.dma_start(wh_t[:], wh_d.ap())
            nc.scalar.dma_start(em_t[:], em_d.ap())
            nc.scalar.dma_start(
                w2_ts[0][:], bass.AP(w2_d, 0, [[3 * w2blk, P], [1, w2blk]]))
            for t in (1, 2):
                nc.sync.dma_start(
                    w2_ts[t][:],
                    bass.AP(w2_d, t * w2blk, [[3 * w2blk, P], [1, w2blk]]))

            # h1 pads must be zero; activation only writes valid 32-col spans.
            nc.vector.memset(h1_t[:], 0.0)
            nc.gpsimd.memset(wz_t[:], 0.0)

            # ---- PE warm-up: release the HAM clock gate during DMA wait --
            ps_warm = psp.tile([P, N1], f32, tag="cva", name="warm")
            for _ in range(16):
                nc.tensor.matmul(ps_warm[:, :], lhsT=wz_t[:, 0:P],
                                 rhs=wz_t[:, :], start=True, stop=True)

            # ---- conv1: 576 -> 256 over 7 rows, arrival-ordered taps -----
            ps1s = [psp.tile([P, N1], f32, tag="cva", name="ps1a"),
                    psp.tile([P, N1], f32, tag="cvb", name="ps1b")]
            first = True
            for tap in (1, 0, 3, 2, 5, 4, 7, 6):
                ky, kx = tap // 3, tap % 3
                off = ky * W36 + kx + 1
                for kc in range(KC1):
                    for mc in range(MC):
                        nc.tensor.matmul(
                            ps1s[mc][:, :],
                            lhsT=w1_ts[tap][:, (kc * MC + mc) * P:
                                            (kc * MC + mc) * P + P],
                            rhs=x_t[:, kc * XBLK + off: kc * XBLK + off + N1],
                            start=first, stop=False,
                        )
                    first = False
            # K=64 chunk (input channels 512..575), all taps from one tile
            for tap in range(9):
                ky, kx = tap // 3, tap % 3
                off = ky * W36 + kx + 1
                for mc in range(MC):
                    nc.tensor.matmul(
                        ps1s[mc][:, :],
                        lhsT=wh_t[0:64, (tap * MC + mc) * P:
                                  (tap * MC + mc) * P + P],
                        rhs=xh_t[0:64, off: off + N1],
                        start=False, stop=False,
                    )
            # tap 8 (arrives last on the sync queue)
            off8 = 2 * W36 + 2 + 1
            for kc in range(KC1):
                for mc in range(MC):
                    nc.tensor.matmul(
                        ps1s[mc][:, :],
                        lhsT=w1_ts[8][:, (kc * MC + mc) * P:
                                      (kc * MC + mc) * P + P],
                        rhs=x_t[:, kc * XBLK + off8: kc * XBLK + off8 + N1],
                        start=False, stop=(kc == KC1 - 1),
                    )

            for mc in range(MC):
                # ReLU(x + b) into the valid 32-wide spans of padded h1 rows
                ps1 = ps1s[mc]
                src = bass.AP(ps1.tensor, ps1.offset,
                              [[N1, P], [W36, R1], [1, 32]])
                dstb = h1_t[:, :]
                dst = bass.AP(dstb.tensor, dstb.offset + mc * H1BLK + 2,
                              [[MC * H1BLK + H1SLACK, P], [W36, R1], [1, 32]])
                nc.scalar.activation(dst, src, Relu, bias=bb_t[:, mc:mc + 1])

            # zero h1 rows outside the global image (cores 0 and 7): per-row
            # mask broadcast over the 36 cols of each row
            for mc in range(MC):
                h1b = h1_t[:, :]
                wmb = wm_t[:, :]
                mask = bass.AP(wmb.tensor, wmb.offset + KC2 * OUT_C,
                               [[KC2 * OUT_C + R1, P], [1, R1], [0, W36]])
                h1ap3 = bass.AP(h1b.tensor, h1b.offset + mc * H1BLK,
                                [[MC * H1BLK + H1SLACK, P], [W36, R1], [1, W36]])
                nc.vector.tensor_mul(h1ap3, h1ap3, mask)

            # ---- conv2: 256 -> 256 over 5 rows, tap-triple arrival order -
            ps2s = [psp.tile([P, N2], f32, tag="cva", name="ps2a"),
                    psp.tile([P, N2], f32, tag="cvb", name="ps2b")]
            n_acc = 9 * KC2
            i_acc = 0
            for tap in range(9):
                ky, kx = tap // 3, tap % 3
                off = ky * W36 + kx + 1
                for kc in range(KC2):
                    for mc in range(MC):
                        nc.tensor.matmul(
                            ps2s[mc][:, :],
                            lhsT=w2_ts[tap // 3][:, ((tap % 3) * KC2 + kc) * MC * P
                                                 + mc * P:
                                                 ((tap % 3) * KC2 + kc) * MC * P
                                                 + mc * P + P],
                            rhs=h1_t[:, kc * H1BLK + off: kc * H1BLK + off + N2],
                            start=(i_acc == 0), stop=(i_acc == n_acc - 1),
                        )
                    i_acc += 1
            for mc in range(MC):
                ps2 = ps2s[mc]
                src2 = bass.AP(ps2.tensor, ps2.offset,
                               [[N2, P], [W36, R2], [1, 32]])
                h2b = h2_t[:, :]
                dst2 = bass.AP(h2b.tensor, h2b.offset + mc * NV2,
                               [[MC * NV2, P], [W36, R2], [1, 32]])
                nc.scalar.activation(dst2, src2, Relu, bias=bb_t[:, 2 + mc:3 + mc])

            # ---- 1x1 conv 256 -> 24 into (w, (h, c)) h-major -------------
            psr = psp.tile([32, R2 * OUT_C], f32, tag="psr")
            for h in range(R2):
                for kc in range(KC2):
                    nc.tensor.matmul(
                        psr[:, h * OUT_C:(h + 1) * OUT_C],
                        lhsT=h2_t[:, kc * NV2 + h * W36: kc * NV2 + h * W36 + 32],
                        rhs=wm_t[:, kc * OUT_C:(kc + 1) * OUT_C],
                        start=(kc == 0), stop=(kc == KC2 - 1),
                    )
            # h-major layout means rt is a plain copy of psr
            nc.vector.tensor_copy(rt_t[:, :], psr[:, :])

            # ---- chunked W expansion + H replication + writes ------------
            # chunk 0: owned rows 0,1 -> SBUF/PSUM partitions 0..47;
            # chunk 1: rows 2,3 -> partitions 64..111 (PE out base must be
            # 0/32/64).  HBM row offset for (h, c) is RUN*OUT_W*(24h + c).
            psw = psp.tile([P, OUT_W], f32, tag="psw")
            psa = psp.tile([P, OUT_W], f32, tag="psa")
            for ch in range(2):
                ps = 64 * ch           # sbuf/psum partition base
                pd = 48 * ch           # HBM row-block base
                lhs_pure = rt_t[:, 48 * ch: 48 * ch + 48]
                lhs_next = rt_t[:, 48 * ch + OUT_C: 48 * ch + OUT_C + 48]
                for j in range(2):
                    nc.tensor.matmul(psw[ps:ps + 48, j * 512:(j + 1) * 512],
                                     lhsT=lhs_pure,
                                     rhs=em_t[:, j * 512:(j + 1) * 512],
                                     start=True, stop=True)
                    nc.tensor.matmul(psa[ps:ps + 48, j * 512:(j + 1) * 512],
                                     lhsT=lhs_pure,
                                     rhs=em_t[:, j * 512:(j + 1) * 512],
                                     start=True, stop=False)
                    nc.tensor.matmul(psa[ps:ps + 48, j * 512:(j + 1) * 512],
                                     lhsT=lhs_next,
                                     rhs=em_t[:, j * 512:(j + 1) * 512],
                                     start=False, stop=True)
                nc.scalar.activation(rw_t[ps:ps + 48, :], psw[ps:ps + 48, :],
                                     Ident, bias=bb_t[ps:ps + 48, 4:5])
                nc.scalar.activation(av_t[ps:ps + 48, :], psa[ps:ps + 48, :],
                                     Ident, scale=0.5,
                                     bias=bb_t[ps:ps + 48, 4:5])
                rwb = rw_t[:, :]
                avb = av_t[:, :]
                srcp = bass.AP(rwb.tensor, rwb.offset + ps * OUT_W,
                               [[OUT_W, 48], [0, 21], [1, OUT_W]])
                srca = bass.AP(avb.tensor, avb.offset + ps * OUT_W,
                               [[OUT_W, 48], [1, OUT_W]])
                dstp = bass.AP(out_d, pd * RUN * OUT_W,
                               [[RUN * OUT_W, 48], [OUT_W, 21], [1, OUT_W]])
                dsta = bass.AP(out_d, pd * RUN * OUT_W + 21 * OUT_W,
                               [[RUN * OUT_W, 48], [1, OUT_W]])
                engp = nc.sync if ch == 0 else nc.scalar
                enga = nc.scalar if ch == 0 else nc.sync
                engp.dma_start(dstp, srcp)
                enga.dma_start(dsta, srca)

    nc.compile()
    return nc


def _pack_inputs(x, w1, b1, w2, b2, wr, br):
    x = np.asarray(x, np.float32)
    w1 = np.asarray(w1, np.float32)
    w2 = np.asarray(w2, np.float32)
    wr = np.asarray(wr, np.float32)
    b1 = np.asarray(b1, np.float32)
    b2 = np.asarray(b2, np.float32)
    br = np.asarray(br, np.float32)

    xv = x[0]  # (576, 32, 32)
    xp = np.zeros((NCORES, P, KC1, RX, W36), np.float16)
    xhp = np.zeros((NCORES, 64, RX, W36), np.float16)
    for k in range(NCORES):
        for r in range(RX):
            g = 4 * k - 2 + r
            if 0 <= g < H:
                blkv = xv[:, g, :]  # (576, 32)
                xp[k, :, :, r, 2:34] = blkv[:512].reshape(KC1, P, W).transpose(1, 0, 2)
                xhp[k, :, r, 2:34] = blkv[512:]
    xp = xp.reshape(NCORES, P, KC1 * XBLK)
    xp = np.concatenate([xp, np.zeros((NCORES, P, XSLACK), np.float16)], axis=2)
    xhp = xhp.reshape(NCORES, 64, XBLK)
    xhp = np.concatenate([xhp, np.zeros((NCORES, 64, XSLACK), np.float16)],
                         axis=2)

    # w1 full chunks: [p, tap, kc, mc, m] = w1[mc*128+m, kc*128+p, ky, kx]
    w1v = w1.transpose(2, 3, 1, 0).reshape(9, IN_C, MID_C)  # (tap, ci, co)
    w1p = (w1v[:, :512, :].reshape(9, KC1, P, MC, P)
           .transpose(2, 0, 1, 3, 4).reshape(P, 9 * KC1 * MC * P))
    w1p = np.ascontiguousarray(w1p, np.float16)
    # w1 K=64 chunk: [p, tap, mc, m] = w1[mc*128+m, 512+p, ky, kx]
    whp = (w1v[:, 512:, :].reshape(9, 64, MC, P)
           .transpose(1, 0, 2, 3).reshape(64, 9 * MC * P))
    whp = np.ascontiguousarray(whp, np.float16)

    w2v = w2.transpose(2, 3, 1, 0).reshape(9, MID_C, MID_C)
    w2p = (w2v.reshape(9, KC2, P, MC, P).transpose(2, 0, 1, 3, 4)
           .reshape(P, 9 * KC2 * MC * P))
    w2p = np.ascontiguousarray(w2p, np.float16)

    wrp = wr.T.reshape(KC2, P, OUT_C).transpose(1, 0, 2).reshape(P, KC2 * OUT_C)
    # row mask for h1 (per-core): mask col j covers h1 row j
    mkp = np.zeros((NCORES, P, R1), np.float16)
    for k in range(NCORES):
        for r in range(R1):
            if 0 <= 4 * k - 1 + r < H:
                mkp[k, :, r] = 1.0
    wmp = np.zeros((NCORES, P, KC2 * OUT_C + R1), np.float16)
    wmp[:, :, :KC2 * OUT_C] = wrp[None]
    wmp[:, :, KC2 * OUT_C:] = mkp

    bbp = np.zeros((P, 5), np.float32)
    bbp[:, 0:2] = b1.reshape(MC, P).T
    bbp[:, 2:4] = b2.reshape(MC, P).T
    # expansion-chunk bias: partitions 0..47 and 64..111 hold br[p % 24]
    bbp[0:48, 4] = np.tile(br, 2)
    bbp[64:112, 4] = np.tile(br, 2)
    em = (np.arange(OUT_W) // 32 == np.arange(32)[:, None]).astype(np.float16)

    shared = dict(w1p=w1p, whp=whp, w2p=w2p, bbp=bbp, emp=em)
    in_maps = []
    for k in range(NCORES):
        m = dict(shared)
        m["xs"] = np.ascontiguousarray(xp[k])
        m["xh"] = np.ascontiguousarray(xhp[k])
        m["wmp"] = np.ascontiguousarray(wmp[k])
        in_maps.append(m)
    return in_maps


def kernel(x, w1, b1, w2, b2, wr, br):
    from concourse.bass_utils import run_bass_kernel_spmd

    if "nc" not in _prog_cache:
        _prog_cache["nc"] = _build_program()
    nc = _prog_cache["nc"]

    in_maps = _pack_inputs(x, w1, b1, w2, b2, wr, br)
    res = run_bass_kernel_spmd(nc, in_maps, list(range(NCORES)))

    _, t = _h_runs()
    out = np.empty((1, OUT_C, OUT_H, OUT_W), np.float32)
    for k in range(NCORES):
        # (4*24*22, 1024) rows ordered (h, c, run) h-major
        buf = res.results[k]["outb"].astype(np.float32)
        buf = buf.reshape(4, OUT_C, RUN, OUT_W)
        for hl in range(4):
            h = 4 * k + hl
            n = t[h + 1] - t[h]
            if h < H - 1:
                out[0, :, t[h]:t[h] + n - 1, :] = buf[hl, :, :n - 1, :]
                out[0, :, t[h] + n - 1, :] = buf[hl, :, RUN - 1, :]
            else:
                out[0, :, t[h]:t[h] + n, :] = buf[hl, :, :n, :]
    return out
